# revision 33
# baseline (speedup 1.0000x reference)
"""Block-sparse top-k masked linear for Trainium2, tensor-parallel over 8 cores.

out = (block_masked x) @ W + bias
  x: (128, 1, 4096) fp16, W: (4096, 11008) fp16, bias: (11008,) fp16
  mask: per (32-row x 64-col) block of x, keep blocks whose mean |x| is
  >= the 32nd-largest of the 64 k-block activations in that row block.

Sharding: column-parallel — each of the 8 cores gets an 11008/8 = 1376
column slice of W and bias; x is replicated; outputs are concatenated.

Schedule: x chunks dispatch first on the two HWDGE queues (the first 8
dispatches = x0-3, cpack, E, w0, w1 exactly fill the 8 DMA semaphore
lanes), then the W stream in big chunks alternating queues. W is
pre-packed on the host to (128, 32*1376): partition p holds row
kt*128+p of every k-tile contiguously, so W chunks move multi-KB
contiguous runs per partition (full HBM rate). 14 junk matmuls at the
start hold the PE busy for a full HAM window so the clock gate opens at
2.4 GHz and the engines are sequenced so the PE never idles a full
window again. The top-k mask is applied post-transpose (keep_scal) and
the transpose -> copy -> mask -> matmul chain is software-pipelined
across engines, transposes running 3 k-tiles ahead of their matmuls.
"""
from contextlib import ExitStack

import numpy as np

import concourse.bass as bass
import concourse.tile as tile
from concourse import bacc, mybir
from concourse.bass_utils import run_bass_kernel_spmd

F16 = mybir.dt.float16
F32 = mybir.dt.float32
AX = mybir.AxisListType
ALU = mybir.AluOpType
ACT = mybir.ActivationFunctionType

M = 128          # rows of x
K = 4096         # contraction
N = 11008        # out features
NCORES = 8
NLOC = N // NCORES           # 1376 columns per core
BLOCK_M, BLOCK_K = 32, 64
NBM, NBK = M // BLOCK_M, K // BLOCK_K   # 4 row blocks, 64 k blocks
KEEP = 32                               # k blocks kept per row block
NKT = K // 128                          # 32 k tiles of 128
N_TILES = [(0, 512), (512, 512), (1024, 352)]   # n-tile offsets/sizes
# W streaming chunks in k-tiles: 2-tile chunks (5.5KB descriptors) so the
# warm GEMM never waits more than ~0.3us for the next chunk (longer PE
# idles re-throttle the HAM clock gate); 1-tile chunks at the tail
W_CHUNKS = [2] * 15 + [1, 1]
assert sum(W_CHUNKS) == NKT
# x as 2 DMAs with 4KB descriptors: big descriptors ride the early DMA
# ramp at full rate, and x clears the queues quickly so the W stream gets
# the whole HBM bandwidth from ~11us on
X_CHUNKS = [2048, 2048]
assert sum(X_CHUNKS) == K
RED_W = 1024                 # block-sum reduce slice width (2 per x chunk)
N_WARM = 14
IDW = 128 + NBM              # ident | E (f16, cast to f32 on chip)
JKW = 128 + NKT              # JH | Ksel (64 partitions only)


def _program(ctx: ExitStack, tc: tile.TileContext, ins, outs):
    nc = tc.nc
    x_d, w_d, cp_d, e_d, b_d = ins
    (o_d,) = outs

    const = ctx.enter_context(tc.tile_pool(name="const", bufs=1))
    mk = ctx.enter_context(tc.tile_pool(name="mk", bufs=1))
    wpool = ctx.enter_context(tc.tile_pool(name="wpool", bufs=1))
    opool = ctx.enter_context(tc.tile_pool(name="opool", bufs=1))
    psum = ctx.enter_context(tc.tile_pool(name="psum", bufs=1, space="PSUM"))

    # ---- x first on both HWDGE queues (it gates the top-k mask), then
    # consts, then the W stream
    NCH = len(X_CHUNKS)
    xc = ctx.enter_context(tc.tile_pool(name="xc", bufs=1))
    x_tiles = []
    x_off = [0]
    for c in range(NCH):
        x_off.append(x_off[-1] + X_CHUNKS[c])
    for c in range(NCH):
        x_c = xc.tile([128, X_CHUNKS[c]], F16, name=f"xch{c}")
        (nc.sync if c % 2 == 0 else nc.scalar).dma_start(
            x_c[:], x_d[:, x_off[c]:x_off[c + 1]])
        x_tiles.append(x_c)
    # kt -> (chunk, col offset within chunk)
    kt_loc = []
    for c in range(NCH):
        for t in range(X_CHUNKS[c] // 128):
            kt_loc.append((c, t * 128))

    # consts, shrunk to their live partitions: identE (128 rows), JH|Ksel
    # (64 rows), bias (1 row); E is f16 in identE and cast to f32 on chip
    idpack = const.tile([128, IDW], F16)
    nc.sync.dma_start(idpack[:], cp_d)
    ident = idpack[:, 0:128]
    jkpack = const.tile([64, JKW], F16)
    nc.scalar.dma_start(jkpack[:], e_d)
    jh = jkpack[:, 0:128]
    ksel = jkpack[:, 128:128 + NKT]
    bias_sb = const.tile([1, NLOC], F16)
    nc.sync.dma_start(bias_sb[:], b_d)

    # W chunks: packed layout w_d[p, kt*NLOC + n] = W[kt*128 + p, n] →
    # per-partition contiguous runs of nkt*2752 bytes per chunk
    w_tiles = []          # per k-tile: (chunk_tile, col offset)
    kt0 = 0
    for ci, nkt_c in enumerate(W_CHUNKS):
        w_t = wpool.tile([128, nkt_c * NLOC], F16, name=f"wch{ci}")
        (nc.sync if ci % 2 == 0 else nc.scalar).dma_start(
            w_t[:], w_d[:, kt0 * NLOC:(kt0 + nkt_c) * NLOC])
        for i in range(nkt_c):
            w_tiles.append((w_t, i * NLOC))
        kt0 += nkt_c

    # ---- HAM warm-up: junk matmuls back-to-back so the PE clock gate
    # opens (a full 3.4us activity window busy) before the real work
    warm_sb = mk.tile([128, 512], F16)
    nc.vector.memset(warm_sb[:], 0.0)
    pbanks = [psum.tile([128, 512], F32, name=f"pn{i}", tag=f"pn{i}")
              for i in range(3)]
    for i in range(N_WARM):
        nc.tensor.matmul(pbanks[0][:], lhsT=warm_sb[:, 0:128], rhs=warm_sb[:],
                         start=True, stop=True)

    # ---- |x| block partial sums (DVE, on slices as chunks land)
    part_n = mk.tile([128, NBK], F32)
    for c in range(NCH):
        for s0 in range(0, X_CHUNKS[c], RED_W):
            # part_n[m, j] = sum_k |x[m, 64 j + k]| over this slice's j's
            j0 = (x_off[c] + s0) // BLOCK_K
            nc.vector.tensor_reduce(
                part_n[:, j0:j0 + RED_W // BLOCK_K],
                x_tiles[c][:, s0:s0 + RED_W]
                    .rearrange("p (j k) -> p j k", k=BLOCK_K),
                axis=AX.X, op=ALU.add, apply_absolute_value=True)

    # ---- first transposes (x-gated only) keep the PE busy after warm-up
    xtpool = ctx.enter_context(tc.tile_pool(name="xtpool", bufs=NKT))
    xmpool = ctx.enter_context(tc.tile_pool(name="xmpool", bufs=NKT))
    tp_tiles = {}
    xt_tiles = {}
    xm_tiles = {}

    def stage_transpose(kt):      # PE
        c, co = kt_loc[kt]
        tp = psum.tile([128, 128], F16, name=f"tp{kt}", tag="tp", bufs=3)
        nc.tensor.transpose(tp[:], x_tiles[c][:, co:co + 128], ident)
        tp_tiles[kt] = tp

    def stage_copy(kt):           # DVE: drain PSUM
        xt_t = xtpool.tile([128, 128], F16, name=f"xt{kt}", tag="xt")
        nc.vector.tensor_copy(xt_t[:], tp_tiles[kt][:])
        xt_tiles[kt] = xt_t

    def stage_xm(kt):             # DVE: apply mask
        xm_t = xmpool.tile([128, 128], F16, name=f"xm{kt}", tag="xm")
        nc.vector.tensor_tensor(
            xm_t[:].rearrange("p (b m) -> p b m", b=NBM),
            xt_tiles[kt][:].rearrange("p (b m) -> p b m", b=NBM),
            keep_scal[:, kt:kt + 97:32].unsqueeze(-1)
                .broadcast_to((128, NBM, BLOCK_M)),
            op=ALU.mult)
        xm_tiles[kt] = xm_t

    stage_transpose(0)
    stage_transpose(1)
    stage_transpose(2)

    # ---- top-k mask chain (fused DVE ops to shorten the critical path)
    # E cast f16 -> f32 (matmul lhsT dtype must match the f32 rhs)
    e_sb = const.tile([128, NBM], F32)
    nc.vector.tensor_copy(e_sb[:], idpack[:, 128:128 + NBM])
    # ba_ps[b, j] = sum_m E[m, b] * part_n[m, j]  (block sums, b on partitions)
    ba_ps = psum.tile([NBM, NBK], F32, tag="mkps", bufs=2)
    nc.tensor.matmul(ba_ps[:], lhsT=e_sb[:], rhs=part_n[:], start=True, stop=True)

    # mean = sum / 2048 (exact power of two), rounded to f16 like jnp.mean
    ba16 = mk.tile([NBM, NBK], F16)
    nc.vector.tensor_scalar_mul(ba16[:], ba_ps[:], 1.0 / 2048.0)

    # acol[i, b] = a[b, i] via PE transpose
    acol_ps = psum.tile([64, NBM], F16, tag="mkps", bufs=2)
    nc.tensor.transpose(acol_ps[:], ba16[:], ident[0:NBM, 0:NBM])
    acol = mk.tile([64, NBM], F16)
    nc.vector.tensor_copy(acol[:], acol_ps[:])

    # arow[i, b*64+j] = a[b, j] on 64 partitions, via block-diag expand +
    # matmul; rhs3[c, b*64+j] = (ba_ps[c, j]/2048 rounded to f16) * [c == b]
    rhs3 = mk.tile([NBM, NBM * NBK], F16)
    nc.vector.tensor_tensor(
        rhs3[:].rearrange("c (b j) -> c b j", b=NBM),
        ba16[:].unsqueeze(1).broadcast_to((NBM, NBM, NBK)),
        ident[0:NBM, 0:NBM].unsqueeze(-1).broadcast_to((NBM, NBM, NBK)),
        op=ALU.mult)
    ones4c = mk.tile([NBM, 64], F16)
    nc.vector.memset(ones4c[:], 1.0)
    arow_ps = psum.tile([64, NBM * NBK], F32, tag="mkps", bufs=2)
    nc.tensor.matmul(arow_ps[:], lhsT=ones4c[:], rhs=rhs3[:], start=True, stop=True)
    arow = mk.tile([64, NBM * NBK], F16)
    nc.vector.tensor_copy(arow[:], arow_ps[:])

    # cnt[i, b] = #{j : a[b, j] > a[b, i]};  keep iff cnt < KEEP
    cmp = mk.tile([64, NBM * NBK], F16)
    nc.vector.tensor_tensor(
        cmp[:].rearrange("i (b j) -> i b j", b=NBM),
        arow[:].rearrange("i (b j) -> i b j", b=NBM),
        acol[:].unsqueeze(-1).broadcast_to((64, NBM, NBK)),
        op=ALU.is_gt)
    cnt = mk.tile([64, NBM], F32)
    nc.vector.tensor_reduce(cnt[:], cmp[:].rearrange("i (b j) -> i b j", b=NBM),
                            axis=AX.X, op=ALU.add)

    # keep_scal[p, b*32+kt] = keep[2kt + p//64, b] via factored selector:
    # rhs2[j, b*32+kt] = [cnt[j, b] < KEEP] * Ksel[j, kt], then JH @ rhs2
    rhs2 = mk.tile([64, 128], F16)
    nc.vector.scalar_tensor_tensor(
        rhs2[:].rearrange("j (b kt) -> j b kt", b=NBM),
        cnt[:].unsqueeze(-1).broadcast_to((64, NBM, NKT)),
        float(KEEP),
        ksel.unsqueeze(1).broadcast_to((64, NBM, NKT)),
        op0=ALU.is_lt, op1=ALU.mult)
    ks_ps = psum.tile([128, 128], F32, tag="mkps", bufs=2)
    nc.tensor.matmul(ks_ps[:], lhsT=jh, rhs=rhs2[:], start=True, stop=True)
    keep_scal = mk.tile([128, 128], F16)
    nc.vector.tensor_copy(keep_scal[:], ks_ps[:])

    ones = const.tile([1, 128], F16)
    nc.vector.memset(ones[:], 1.0)

    # ---- main GEMM: out[m, n] = sum_kt xm_kt.T @ w_kt + ones.T @ bias,
    # software-pipelined: transpose(kt+3) on PE, copy(kt+2)+mask(kt+2) on
    # DVE, matmuls(kt) on PE
    # bias as the FIRST accumulation into each bank (start=True) so the
    # banks are complete right when the last k-tile matmul lands
    for nt, (n0, nsz) in enumerate(N_TILES):
        nc.tensor.matmul(pbanks[nt][:, :nsz], lhsT=ones[:],
                         rhs=bias_sb[:, n0:n0 + nsz], start=True, stop=False)
    stage_copy(0)
    stage_xm(0)
    stage_copy(1)
    stage_xm(1)
    for kt in range(NKT):
        if kt + 2 < NKT:
            stage_copy(kt + 2)
            stage_xm(kt + 2)
        if kt + 3 < NKT:
            stage_transpose(kt + 3)
        w_t, co = w_tiles[kt]
        for nt, (n0, nsz) in enumerate(N_TILES):
            nc.tensor.matmul(pbanks[nt][:, :nsz],
                             lhsT=xm_tiles[kt][:],
                             rhs=w_t[:, co + n0:co + n0 + nsz],
                             start=False, stop=(kt == NKT - 1))
    out_sb = opool.tile([128, NLOC], F16)
    out_dma = [nc.sync, nc.scalar, nc.sync]
    for nt, (n0, nsz) in enumerate(N_TILES):
        src = pbanks[nt][:, :nsz]
        dst = out_sb[:, n0:n0 + nsz]
        if nt % 2 == 0:
            nc.scalar.activation(dst, src, ACT.Copy)
        else:
            nc.vector.tensor_copy(dst, src)
        out_dma[nt].dma_start(o_d[:, n0:n0 + nsz], dst)


_CACHE = {}


def _build():
    if "nc" in _CACHE:
        return _CACHE["nc"]
    nc = bacc.Bacc("TRN2", target_bir_lowering=False, debug=False,
                   num_devices=NCORES)
    x_d = nc.dram_tensor("x", (M, K), F16, kind="ExternalInput").ap()
    w_d = nc.dram_tensor("w", (128, NKT * NLOC), F16, kind="ExternalInput").ap()
    cp_d = nc.dram_tensor("idpack", (128, IDW), F16, kind="ExternalInput").ap()
    e_d = nc.dram_tensor("jkpack", (64, JKW), F16, kind="ExternalInput").ap()
    b_d = nc.dram_tensor("bias", (1, NLOC), F16, kind="ExternalInput").ap()
    o_d = nc.dram_tensor("out", (M, NLOC), F16, kind="ExternalOutput").ap()
    with tile.TileContext(nc) as tc:
        with ExitStack() as ctx:
            _program(ctx, tc, [x_d, w_d, cp_d, e_d, b_d], [o_d])
    nc.compile()
    _CACHE["nc"] = nc
    return nc


def _make_in_maps(x2, weight, bias):
    j_idx = np.arange(64)
    id_np = np.zeros((M, IDW), np.float16)
    id_np[:, 0:128] = np.eye(128, dtype=np.float16)
    for b in range(NBM):
        id_np[b * BLOCK_M:(b + 1) * BLOCK_M, 128 + b] = 1.0
    jk_np = np.zeros((64, JKW), np.float16)
    jk_np[:, 0:128] = (
        j_idx[:, None] % 2 == (np.arange(128)[None, :] // 64)).astype(np.float16)
    jk_np[:, 128:128 + NKT] = (
        j_idx[:, None] // 2 == np.arange(NKT)[None, :]).astype(np.float16)

    weight = np.asarray(weight).astype(np.float16, copy=False)
    bias = np.asarray(bias).astype(np.float16, copy=False)
    in_maps = []
    for c in range(NCORES):
        sl = slice(c * NLOC, (c + 1) * NLOC)
        # pack so partition p holds row kt*128+p of every k-tile contiguously
        w_c = weight[:, sl].reshape(NKT, 128, NLOC).transpose(1, 0, 2)
        in_maps.append({
            "x": x2,
            "w": np.ascontiguousarray(w_c).reshape(128, NKT * NLOC),
            "idpack": id_np,
            "jkpack": jk_np,
            "bias": np.ascontiguousarray(bias[sl].reshape(1, NLOC)),
        })
    return in_maps


def kernel(x: np.ndarray, weight: np.ndarray, bias: np.ndarray) -> np.ndarray:
    x = np.asarray(x)
    weight = np.asarray(weight)
    bias = np.asarray(bias)
    bsz, seq, hidden = x.shape
    assert (bsz, seq, hidden) == (M, 1, K) and weight.shape == (K, N)

    x2 = np.ascontiguousarray(x.reshape(M, K).astype(np.float16, copy=False))
    in_maps = _make_in_maps(x2, weight, bias)
    nc = _build()
    res = run_bass_kernel_spmd(nc, in_maps, core_ids=list(range(NCORES)))
    out = np.concatenate([r["out"] for r in res.results], axis=1)
    return out.reshape(M, 1, N).astype(x.dtype, copy=False)


if __name__ == "__main__":
    rng = np.random.default_rng(0)
    x = rng.standard_normal((M, 1, K)).astype(np.float16)
    w = (rng.standard_normal((K, N)) * 0.01).astype(np.float16)
    b = np.zeros((N,), np.float16)
    out = kernel(x, w, b)
    print(out.shape, out.dtype)


# revision 39
# speedup vs baseline: 1.0850x; 1.0850x over previous
"""Block-sparse top-k masked linear for Trainium2, tensor-parallel over 8 cores.

out = (block_masked x) @ W + bias
  x: (128, 1, 4096) fp16, W: (4096, 11008) fp16, bias: (11008,) fp16
  mask: per (32-row x 64-col) block of x, keep blocks whose mean |x| is
  >= the 32nd-largest of the 64 k-block activations in that row block.

Sharding: column-parallel — each of the 8 cores gets an 11008/8 = 1376
column slice of W and bias; x is replicated; outputs are concatenated.

Schedule: x chunks dispatch first on the two HWDGE queues (the first 8
dispatches = x0-3, cpack, E, w0, w1 exactly fill the 8 DMA semaphore
lanes), then the W stream in big chunks alternating queues. W is
pre-packed on the host to (128, 32*1376): partition p holds row
kt*128+p of every k-tile contiguously, so W chunks move multi-KB
contiguous runs per partition (full HBM rate). 14 junk matmuls at the
start hold the PE busy for a full HAM window so the clock gate opens at
2.4 GHz and the engines are sequenced so the PE never idles a full
window again. The top-k mask is applied post-transpose (keep_scal) and
the transpose -> copy -> mask -> matmul chain is software-pipelined
across engines, transposes running 3 k-tiles ahead of their matmuls.
"""
from contextlib import ExitStack

import numpy as np

import concourse.bass as bass
import concourse.tile as tile
from concourse import bacc, mybir
from concourse.bass_utils import run_bass_kernel_spmd

F16 = mybir.dt.float16
F32 = mybir.dt.float32
AX = mybir.AxisListType
ALU = mybir.AluOpType
ACT = mybir.ActivationFunctionType

M = 128          # rows of x
K = 4096         # contraction
N = 11008        # out features
NCORES = 8
NLOC = N // NCORES           # 1376 columns per core
BLOCK_M, BLOCK_K = 32, 64
NBM, NBK = M // BLOCK_M, K // BLOCK_K   # 4 row blocks, 64 k blocks
KEEP = 32                               # k blocks kept per row block
NKT = K // 128                          # 32 k tiles of 128
N_TILES = [(0, 512), (512, 512), (1024, 352)]   # n-tile offsets/sizes
# W streaming chunks in k-tiles: 2-tile chunks (5.5KB descriptors) so the
# warm GEMM never waits more than ~0.3us for the next chunk (longer PE
# idles re-throttle the HAM clock gate); 1-tile chunks at the tail
W_CHUNKS = [2] * 15 + [1, 1]
assert sum(W_CHUNKS) == NKT
# x as 2 DMAs with 4KB descriptors: big descriptors ride the early DMA
# ramp at full rate, and x clears the queues quickly so the W stream gets
# the whole HBM bandwidth from ~11us on
X_CHUNKS = [2048, 2048]
assert sum(X_CHUNKS) == K
RED_W = 1024                 # block-sum reduce slice width (2 per x chunk)
N_WARM = 14
CPW = 128 + 128 + NKT + NLOC   # packed const: ident | JH | Ksel | bias


def _program(ctx: ExitStack, tc: tile.TileContext, ins, outs):
    nc = tc.nc
    x_d, w_d, cp_d, e_d = ins
    (o_d,) = outs

    const = ctx.enter_context(tc.tile_pool(name="const", bufs=1))
    mk = ctx.enter_context(tc.tile_pool(name="mk", bufs=1))
    wpool = ctx.enter_context(tc.tile_pool(name="wpool", bufs=1))
    opool = ctx.enter_context(tc.tile_pool(name="opool", bufs=1))
    psum = ctx.enter_context(tc.tile_pool(name="psum", bufs=1, space="PSUM"))

    # ---- x first on both HWDGE queues (it gates the top-k mask), then
    # consts, then the W stream
    NCH = len(X_CHUNKS)
    xc = ctx.enter_context(tc.tile_pool(name="xc", bufs=1))
    x_tiles = []
    x_off = [0]
    for c in range(NCH):
        x_off.append(x_off[-1] + X_CHUNKS[c])
    for c in range(NCH):
        x_c = xc.tile([128, X_CHUNKS[c]], F16, name=f"xch{c}")
        (nc.sync if c % 2 == 0 else nc.scalar).dma_start(
            x_c[:], x_d[:, x_off[c]:x_off[c + 1]])
        x_tiles.append(x_c)
    # kt -> (chunk, col offset within chunk)
    kt_loc = []
    for c in range(NCH):
        for t in range(X_CHUNKS[c] // 128):
            kt_loc.append((c, t * 128))

    # consts: one packed f16 DMA (ident | JH | Ksel | bias) + E (f32)
    cpack = const.tile([128, CPW], F16)
    nc.sync.dma_start(cpack[:], cp_d)
    ident = cpack[:, 0:128]
    jh = cpack[0:64, 128:256]
    ksel = cpack[0:64, 256:256 + NKT]
    bias_sb = cpack[0:1, 256 + NKT:256 + NKT + NLOC]
    e_sb = const.tile([128, NBM], F32)
    nc.scalar.dma_start(e_sb[:], e_d)

    # W chunks: packed layout w_d[p, kt*NLOC + n] = W[kt*128 + p, n] →
    # per-partition contiguous runs of nkt*2752 bytes per chunk
    w_tiles = []          # per k-tile: (chunk_tile, col offset)
    kt0 = 0
    for ci, nkt_c in enumerate(W_CHUNKS):
        w_t = wpool.tile([128, nkt_c * NLOC], F16, name=f"wch{ci}")
        (nc.sync if ci % 2 == 0 else nc.scalar).dma_start(
            w_t[:], w_d[:, kt0 * NLOC:(kt0 + nkt_c) * NLOC])
        for i in range(nkt_c):
            w_tiles.append((w_t, i * NLOC))
        kt0 += nkt_c

    # ---- HAM warm-up: junk matmuls back-to-back so the PE clock gate
    # opens (a full 3.4us activity window busy) before the real work
    warm_sb = mk.tile([128, 512], F16)
    nc.vector.memset(warm_sb[:], 0.0)
    pbanks = [psum.tile([128, 512], F32, name=f"pn{i}", tag=f"pn{i}")
              for i in range(3)]
    for i in range(N_WARM):
        nc.tensor.matmul(pbanks[0][:], lhsT=warm_sb[:, 0:128], rhs=warm_sb[:],
                         start=True, stop=True)

    # ---- |x| block partial sums (DVE, on slices as chunks land)
    part_n = mk.tile([128, NBK], F32)
    for c in range(NCH):
        for s0 in range(0, X_CHUNKS[c], RED_W):
            # part_n[m, j] = sum_k |x[m, 64 j + k]| over this slice's j's
            j0 = (x_off[c] + s0) // BLOCK_K
            nc.vector.tensor_reduce(
                part_n[:, j0:j0 + RED_W // BLOCK_K],
                x_tiles[c][:, s0:s0 + RED_W]
                    .rearrange("p (j k) -> p j k", k=BLOCK_K),
                axis=AX.X, op=ALU.add, apply_absolute_value=True)

    # ---- first transposes (x-gated only) keep the PE busy after warm-up
    xtpool = ctx.enter_context(tc.tile_pool(name="xtpool", bufs=NKT))
    xmpool = ctx.enter_context(tc.tile_pool(name="xmpool", bufs=NKT))
    tp_tiles = {}
    xt_tiles = {}
    xm_tiles = {}

    def stage_transpose(kt):      # PE
        c, co = kt_loc[kt]
        tp = psum.tile([128, 128], F16, name=f"tp{kt}", tag="tp", bufs=3)
        nc.tensor.transpose(tp[:], x_tiles[c][:, co:co + 128], ident)
        tp_tiles[kt] = tp

    def stage_copy(kt):           # DVE: drain PSUM
        xt_t = xtpool.tile([128, 128], F16, name=f"xt{kt}", tag="xt")
        nc.vector.tensor_copy(xt_t[:], tp_tiles[kt][:])
        xt_tiles[kt] = xt_t

    def stage_xm(kt):             # DVE: apply mask
        xm_t = xmpool.tile([128, 128], F16, name=f"xm{kt}", tag="xm")
        nc.vector.tensor_tensor(
            xm_t[:].rearrange("p (b m) -> p b m", b=NBM),
            xt_tiles[kt][:].rearrange("p (b m) -> p b m", b=NBM),
            keep_scal[:, kt:kt + 97:32].unsqueeze(-1)
                .broadcast_to((128, NBM, BLOCK_M)),
            op=ALU.mult)
        xm_tiles[kt] = xm_t

    stage_transpose(0)
    stage_transpose(1)
    stage_transpose(2)

    # ---- top-k mask chain (fused DVE ops to shorten the critical path)
    # ba_ps[b, j] = sum_m E[m, b] * part_n[m, j]  (block sums, b on partitions)
    ba_ps = psum.tile([NBM, NBK], F32, tag="mkps", bufs=2)
    nc.tensor.matmul(ba_ps[:], lhsT=e_sb[:], rhs=part_n[:], start=True, stop=True)

    # mean = sum / 2048 (exact power of two), rounded to f16 like jnp.mean
    ba16 = mk.tile([NBM, NBK], F16)
    nc.vector.tensor_scalar_mul(ba16[:], ba_ps[:], 1.0 / 2048.0)

    # acol[i, b] = a[b, i] via PE transpose
    acol_ps = psum.tile([64, NBM], F16, tag="mkps", bufs=2)
    nc.tensor.transpose(acol_ps[:], ba16[:], ident[0:NBM, 0:NBM])
    acol = mk.tile([64, NBM], F16)
    nc.vector.tensor_copy(acol[:], acol_ps[:])

    # arow[i, b*64+j] = a[b, j] on 64 partitions, via block-diag expand +
    # matmul; rhs3[c, b*64+j] = (ba_ps[c, j]/2048 rounded to f16) * [c == b]
    rhs3 = mk.tile([NBM, NBM * NBK], F16)
    nc.vector.tensor_tensor(
        rhs3[:].rearrange("c (b j) -> c b j", b=NBM),
        ba16[:].unsqueeze(1).broadcast_to((NBM, NBM, NBK)),
        ident[0:NBM, 0:NBM].unsqueeze(-1).broadcast_to((NBM, NBM, NBK)),
        op=ALU.mult)
    ones4c = mk.tile([NBM, 64], F16)
    nc.vector.memset(ones4c[:], 1.0)
    arow_ps = psum.tile([64, NBM * NBK], F32, tag="mkps", bufs=2)
    nc.tensor.matmul(arow_ps[:], lhsT=ones4c[:], rhs=rhs3[:], start=True, stop=True)
    arow = mk.tile([64, NBM * NBK], F16)
    nc.vector.tensor_copy(arow[:], arow_ps[:])

    # cnt[i, b] = #{j : a[b, j] > a[b, i]};  keep iff cnt < KEEP
    cmp = mk.tile([64, NBM * NBK], F16)
    nc.vector.tensor_tensor(
        cmp[:].rearrange("i (b j) -> i b j", b=NBM),
        arow[:].rearrange("i (b j) -> i b j", b=NBM),
        acol[:].unsqueeze(-1).broadcast_to((64, NBM, NBK)),
        op=ALU.is_gt)
    cnt = mk.tile([64, NBM], F32)
    nc.vector.tensor_reduce(cnt[:], cmp[:].rearrange("i (b j) -> i b j", b=NBM),
                            axis=AX.X, op=ALU.add)

    # keep_scal[p, b*32+kt] = keep[2kt + p//64, b] via factored selector:
    # rhs2[j, b*32+kt] = [cnt[j, b] < KEEP] * Ksel[j, kt], then JH @ rhs2
    rhs2 = mk.tile([64, 128], F16)
    nc.vector.scalar_tensor_tensor(
        rhs2[:].rearrange("j (b kt) -> j b kt", b=NBM),
        cnt[:].unsqueeze(-1).broadcast_to((64, NBM, NKT)),
        float(KEEP),
        ksel.unsqueeze(1).broadcast_to((64, NBM, NKT)),
        op0=ALU.is_lt, op1=ALU.mult)
    ks_ps = psum.tile([128, 128], F32, tag="mkps", bufs=2)
    nc.tensor.matmul(ks_ps[:], lhsT=jh, rhs=rhs2[:], start=True, stop=True)
    keep_scal = mk.tile([128, 128], F16)
    nc.vector.tensor_copy(keep_scal[:], ks_ps[:])

    ones = const.tile([1, 128], F16)
    nc.vector.memset(ones[:], 1.0)

    # ---- main GEMM: out[m, n] = sum_kt xm_kt.T @ w_kt + ones.T @ bias,
    # software-pipelined: transpose(kt+3) on PE, copy(kt+2)+mask(kt+2) on
    # DVE, matmuls(kt) on PE
    # bias as the FIRST accumulation into each bank (start=True) so the
    # banks are complete right when the last k-tile matmul lands
    for nt, (n0, nsz) in enumerate(N_TILES):
        nc.tensor.matmul(pbanks[nt][:, :nsz], lhsT=ones[:],
                         rhs=bias_sb[:, n0:n0 + nsz], start=True, stop=False)
    stage_copy(0)
    stage_xm(0)
    stage_copy(1)
    stage_xm(1)
    for kt in range(NKT):
        if kt + 2 < NKT:
            stage_copy(kt + 2)
            stage_xm(kt + 2)
        if kt + 3 < NKT:
            stage_transpose(kt + 3)
        w_t, co = w_tiles[kt]
        for nt, (n0, nsz) in enumerate(N_TILES):
            nc.tensor.matmul(pbanks[nt][:, :nsz],
                             lhsT=xm_tiles[kt][:],
                             rhs=w_t[:, co + n0:co + n0 + nsz],
                             start=False, stop=(kt == NKT - 1))
    out_sb = opool.tile([128, NLOC], F16)
    out_dma = [nc.sync, nc.scalar, nc.sync]
    for nt, (n0, nsz) in enumerate(N_TILES):
        src = pbanks[nt][:, :nsz]
        dst = out_sb[:, n0:n0 + nsz]
        if nt % 2 == 0:
            nc.scalar.activation(dst, src, ACT.Copy)
        else:
            nc.vector.tensor_copy(dst, src)
        out_dma[nt].dma_start(o_d[:, n0:n0 + nsz], dst)


_CACHE = {}


def _build():
    if "nc" in _CACHE:
        return _CACHE["nc"]
    nc = bacc.Bacc("TRN2", target_bir_lowering=False, debug=False,
                   num_devices=NCORES)
    x_d = nc.dram_tensor("x", (M, K), F16, kind="ExternalInput").ap()
    w_d = nc.dram_tensor("w", (128, NKT * NLOC), F16, kind="ExternalInput").ap()
    cp_d = nc.dram_tensor("cpack", (128, CPW), F16, kind="ExternalInput").ap()
    e_d = nc.dram_tensor("E", (M, NBM), F32, kind="ExternalInput").ap()
    o_d = nc.dram_tensor("out", (M, NLOC), F16, kind="ExternalOutput").ap()
    with tile.TileContext(nc) as tc:
        with ExitStack() as ctx:
            _program(ctx, tc, [x_d, w_d, cp_d, e_d], [o_d])
    nc.compile()
    _CACHE["nc"] = nc
    return nc


def _make_in_maps(x2, weight, bias):
    e_np = np.zeros((M, NBM), np.float32)
    for b in range(NBM):
        e_np[b * BLOCK_M:(b + 1) * BLOCK_M, b] = 1.0
    j_idx = np.arange(64)
    cp_np = np.zeros((M, CPW), np.float16)
    cp_np[:, 0:128] = np.eye(128, dtype=np.float16)
    cp_np[0:64, 128:256] = (
        j_idx[:, None] % 2 == (np.arange(128)[None, :] // 64)).astype(np.float16)
    cp_np[0:64, 256:256 + NKT] = (
        j_idx[:, None] // 2 == np.arange(NKT)[None, :]).astype(np.float16)

    weight = np.asarray(weight).astype(np.float16, copy=False)
    bias = np.asarray(bias).astype(np.float16, copy=False)
    in_maps = []
    for c in range(NCORES):
        sl = slice(c * NLOC, (c + 1) * NLOC)
        cp_c = cp_np.copy()
        cp_c[0, 256 + NKT:256 + NKT + NLOC] = bias[sl]
        # pack so partition p holds row kt*128+p of every k-tile contiguously
        w_c = weight[:, sl].reshape(NKT, 128, NLOC).transpose(1, 0, 2)
        in_maps.append({
            "x": x2,
            "w": np.ascontiguousarray(w_c).reshape(128, NKT * NLOC),
            "cpack": cp_c,
            "E": e_np,
        })
    return in_maps


def kernel(x: np.ndarray, weight: np.ndarray, bias: np.ndarray) -> np.ndarray:
    x = np.asarray(x)
    weight = np.asarray(weight)
    bias = np.asarray(bias)
    bsz, seq, hidden = x.shape
    assert (bsz, seq, hidden) == (M, 1, K) and weight.shape == (K, N)

    x2 = np.ascontiguousarray(x.reshape(M, K).astype(np.float16, copy=False))
    in_maps = _make_in_maps(x2, weight, bias)
    nc = _build()
    res = run_bass_kernel_spmd(nc, in_maps, core_ids=list(range(NCORES)))
    out = np.concatenate([r["out"] for r in res.results], axis=1)
    return out.reshape(M, 1, N).astype(x.dtype, copy=False)


if __name__ == "__main__":
    rng = np.random.default_rng(0)
    x = rng.standard_normal((M, 1, K)).astype(np.float16)
    w = (rng.standard_normal((K, N)) * 0.01).astype(np.float16)
    b = np.zeros((N,), np.float16)
    out = kernel(x, w, b)
    print(out.shape, out.dtype)
